# revision 72
# baseline (speedup 1.0000x reference)
"""Trainium2 Bass kernel for MQA cross-attention (nn_CrossAttention).

Reference computation (fp32):
    q = (x @ Wq).reshape(b, n, 16, 128).transpose(0,2,1,3) * 128**-0.5
    sim = q @ k^T   (k/v shared across heads, MQA)
    out = softmax(sim) @ v
    y = out.merge_heads @ Wo
Tolerance is rel-err < 2e-2 vs the fp32 reference; the fp16 datapath below
lands ~6e-4.

Sharding: pure sequence-parallel across 8 cores. Each core gets 256 rows
of x per batch (512 rows total), full Wq/Wo/k/v, and produces its 512 rows
of the output. No collectives, no host-side reduction.

PE floor: 4 GEMM stages x 4.3 GFLOP/core = 524288 matmul rows at
1 cycle/row = 218.6us; this kernel models ~231.3us (94.5% PE busy).

All matmul operands are fp16: same PE rate as f32r (1 cycle/row), half
the DMA bytes, and 2-byte dtypes unlock the DVE 2x mode for the softmax
row-sum accumulation.  PSUM stays fp32; the softmax denominator tail
(fold / partition all-reduce / reciprocal / normalize) stays fp32.
Wq and v are host-relaid so every DMA line is >=1KB-contiguous per
partition (strided 256B lines run ~2x slower).

Per-core schedule:
  Prologue: xt/Wq(h0,h1) DMA'd in need-order; the et13-15 Wq slice and
    k/v for batch 0 ride behind the xt stream, landing just-in-time in
    its shadow (each DMA's completion costs a fixed ~900ns semaphore
    propagation, so the critical prefix is kept minimal); dummy
    matmuls on a memset tile warm the PE p-state ramp (2.4GHz only after
    3us continuous busy) inside the DMA-bound wait; heads 0/1 q-project
    et-interleaved, DMA-paced.
  16 attention units (hp-major, b inner), each:
    - software-pipelined jg loop: exp(jg) [ACT, PSUM->fp16 SBUF] is
      chased by sims(jg+1) so the exp->av and sim->exp latencies overlap
      PE work; 16 q-projection matmuls of head u+2 interleave with steps
      (4,3,3,2,2,2,0,0) so the head's PSUM->SBUF copy (DVE) lands by jg5,
      before the next unit's sims need it,
    - DVE accumulates fp16 row-sum partials (2x mode), folds to fp32;
      GpSimd does the 128-way partition reduce; DVE reciprocal +
      normalize (fp32 acc * 1/s -> fp16 outn),
    - Wo prefetches into SBUF 2 ft-tiles/unit during units 0..7.
  Units 14/15 have no q-projection left (ACT would pace them): they run
    the first phase-C batch-0 tiles instead — unit 14 accumulates
    ft 0..13 of tile (b0,rt0,ec0) in two bursts at jg1/jg3 (heads 14/15's
    outn don't exist yet),
    unit 15 closes that group and runs tiles (b0,rt1,ec0) and
    (b0,rt0,ec1), covering the wait for unit 15's softmax tail.
  Phase C: pure-PE output projection from SBUF-resident Wo into the
    qp PSUM pool (freed early; the attention acc banks free too late),
    fp32 out; the final tile is split 256/128/128 so the closing
    copy+store tail is short.
"""

import sys
import numpy as np

for _p in ("/opt/trn_rl_repo", "/root/.axon_site/_ro/trn_rl_repo"):
    if _p not in sys.path:
        sys.path.append(_p)

import concourse.bass as bass  # noqa: E402
import concourse.mybir as mybir  # noqa: E402
import concourse.tile as tile  # noqa: E402
from concourse import bacc, bass_isa  # noqa: E402
from concourse.bass_utils import run_bass_kernel_spmd  # noqa: E402

F32 = mybir.dt.float32
F16 = mybir.dt.float16

B = 2
N = 2048          # query length (global)
J = 2048          # kv length
E = 2048          # model dim
HEADS = 16
DH = 128          # head dim
NCORES = 8
NC_ROWS = N // NCORES        # 256 query rows per core per batch
R = B * NC_ROWS              # 512 rows per core, col = b*NC_ROWS + i
ET = E // 128                # 16 e-tiles
FT = HEADS                   # 16 f-tiles (one per head, DH == 128)
JT = J // 128                # 16 j-tiles
SCALE = float(DH) ** -0.5

_CACHE = {}


def _build(reps: int = 1):
    nc = bacc.Bacc(name=f"mqa_xattn_r{reps}")
    # wq/v are host-relaid so every DMA line is >=4KB contiguous per
    # partition (strided 256B lines run ~2x slower)
    xt_d = nc.declare_dram_parameter("xt", [E, R], F16, isOutput=False)
    kt_d = nc.declare_dram_parameter("kt", [B, DH, J], F16, isOutput=False)
    v_d = nc.declare_dram_parameter("v", [128, B, JT, DH], F16,
                                    isOutput=False)
    wq_d = nc.declare_dram_parameter("wq", [128, HEADS, ET, 128], F16,
                                     isOutput=False)
    wo_d = nc.declare_dram_parameter("wo", [E, E], F16, isOutput=False)
    o_d = nc.declare_dram_parameter("o", [R, E], F32, isOutput=True)

    with tile.TileContext(nc) as tc:
        for _ in range(reps):
            _emit_once(nc, tc, xt_d, kt_d, v_d, wq_d, wo_d, o_d)

    nc.compile()
    return nc


def _emit_once(nc, tc, xt_d, kt_d, v_d, wq_d, wo_d, o_d):
    with tc.tile_pool(name="persist", bufs=1) as pp:
        kt_sb = pp.tile([128, B, J], F16)
        v_sb = pp.tile([128, B, JT, DH], F16)
        qt_all = pp.tile([128, FT, R], F16)
        # free layout: [b][h][i] with i contiguous per head
        outn_all = pp.tile([128, B, FT * NC_ROWS], F16)
        # Wo resident in SBUF: [d-partition][ft][e]; loaded during phase B
        wo_sb = pp.tile([128, FT, E], F16)

        with tc.tile_pool(name="qp_ps", bufs=2, space="PSUM") as qp_ps, \
             tc.tile_pool(name="ost_pool", bufs=6) as ostp:
          with tc.tile_pool(name="xt_pool", bufs=1) as xtp, \
             tc.tile_pool(name="wq_pool", bufs=4) as wqp, \
             tc.tile_pool(name="es_pool", bufs=8) as esp, \
             tc.tile_pool(name="rb_pool", bufs=3) as rbp, \
             tc.tile_pool(name="sg_ps", bufs=2, space="PSUM") as sg_ps, \
             tc.tile_pool(name="acc_ps", bufs=2, space="PSUM") as acc_ps:
            xt_sb = xtp.tile([128, ET, R], F16)

            xt_r = xt_d.rearrange("(et p) r -> p et r", p=128)
            kt_r = kt_d.rearrange("b p j -> p b j")
            wo_r = wo_d.rearrange("(ft p) e -> p ft e", p=128)

            wq_tiles = []

            def load_wq(h, et0=0, et1=ET):
                if et0 == 0:
                    wq_sb = wqp.tile([128, ET, 128], F16, tag="wq",
                                     name=f"wq_sb{h}")
                    wq_tiles.append(wq_sb)
                wq_sb = wq_tiles[h]
                nc.sync.dma_start(wq_sb[:, et0:et1, :],
                                  wq_d[:, h, et0:et1, :])
                return wq_sb

            # DMA order: each transfer is as large as possible (completion
            # only becomes visible ~725ns after the data lands, so many
            # small DMAs serialize on descriptor retirement).  Heads 0+1 of
            # Wq ride in two combined chunks interleaved with the xt
            # stream; kt/v for batch 0 land just before the first
            # attention unit needs them.
            wq01 = wqp.tile([128, 2, ET, 128], F16, tag="wq01", bufs=1)
            wq_tiles.append(wq01[:, 0])
            wq_tiles.append(wq01[:, 1])
            nc.sync.dma_start(wq01[:, :, 0:4, :], wq_d[:, 0:2, 0:4, :])
            nc.sync.dma_start(xt_sb[:, 0:4, :], xt_r[:, 0:4, :])
            nc.sync.dma_start(wq01[:, :, 4:13, :], wq_d[:, 0:2, 4:13, :])
            nc.sync.dma_start(xt_sb[:, 4:8, :], xt_r[:, 4:8, :])
            nc.sync.dma_start(xt_sb[:, 8:12, :], xt_r[:, 8:12, :])
            nc.sync.dma_start(xt_sb[:, 12:ET, :], xt_r[:, 12:ET, :])
            nc.sync.dma_start(wq01[:, :, 13:ET, :], wq_d[:, 0:2, 13:ET, :])
            nc.sync.dma_start(kt_sb[:, 0, 0:1024], kt_r[:, 0, 0:1024])
            load_wq(2, 0, 8)
            nc.sync.dma_start(v_sb[:, 0, 0:4, :], v_d[:, 0, 0:4, :])
            nc.sync.dma_start(kt_sb[:, 0, 1024:J], kt_r[:, 0, 1024:J])
            load_wq(2, 8, ET)
            nc.sync.dma_start(v_sb[:, 0, 4:JT, :], v_d[:, 0, 4:JT, :])
            nc.sync.dma_start(kt_sb[:, 1, :], kt_r[:, 1, :])
            nc.sync.dma_start(v_sb[:, 1, :, :], v_d[:, 1, :, :])

            # PE p-state warmup: the Tensor engine ramps to full clock only
            # after ~3us of continuous execution.  The prologue is DMA-bound
            # anyway, so burn the wait on dummy matmuls over a memset tile —
            # the first real matmuls then run at full speed.
            warm_sb = xtp.tile([128, 512], F16, tag="warm", bufs=1)
            nc.vector.memset(warm_sb[:], 0.0)
            warm_ps = qp_ps.tile([128, R], F32, tag="qp", name="warm_ps")
            for i in range(10):
                nc.tensor.matmul(warm_ps[:], warm_sb[:, 0:128], warm_sb[:],
                                 start=(i == 0), stop=(i == 9))

            # prologue: heads 0/1 q-projections, et-interleaved so PE
            # starts as soon as xt tile 0 lands (DMA-paced)
            q_ps0 = qp_ps.tile([128, R], F32, tag="qp")
            q_ps1 = qp_ps.tile([128, R], F32, tag="qp")
            for et in range(13):
                nc.tensor.matmul(q_ps0[:], wq_tiles[0][:, et, :],
                                 xt_sb[:, et, :],
                                 start=(et == 0), stop=False)
                nc.tensor.matmul(q_ps1[:], wq_tiles[1][:, et, :],
                                 xt_sb[:, et, :],
                                 start=(et == 0), stop=False)
            # tail de-interleaved: h0's group closes ~3 matmuls sooner, so
            # its copy (and the split first sims) start earlier
            for et in range(13, ET):
                nc.tensor.matmul(q_ps0[:], wq_tiles[0][:, et, :],
                                 xt_sb[:, et, :],
                                 start=False, stop=(et == ET - 1))
            for et in range(13, ET):
                nc.tensor.matmul(q_ps1[:], wq_tiles[1][:, et, :],
                                 xt_sb[:, et, :],
                                 start=False, stop=(et == ET - 1))
            # batch-0 halves first (unit 0 reads only columns 0:256), head-0
            # on DVE, head-1 on ACT so nothing serializes on one engine
            with nc.allow_low_precision(reason="f16 q"):
                nc.vector.tensor_copy(qt_all[:, 0, 0:NC_ROWS],
                                      q_ps0[:, 0:NC_ROWS])
            nc.scalar.copy(qt_all[:, 1, 0:NC_ROWS], q_ps1[:, 0:NC_ROWS])
            with nc.allow_low_precision(reason="f16 q"):
                nc.vector.tensor_copy(qt_all[:, 0, NC_ROWS:R],
                                      q_ps0[:, NC_ROWS:R])
            nc.scalar.copy(qt_all[:, 1, NC_ROWS:R], q_ps1[:, NC_ROWS:R])

            # q-projection emission for heads 2.. is spread through the
            # attention units, 2 matmuls per jg iteration.
            q_state = {"h": None, "ps": None, "et": 0}

            def qproj_start(h):
                q_state["h"] = h
                q_state["et"] = 0
                q_state["ps"] = qp_ps.tile([128, R], F32, tag="qp",
                                           name=f"q_ps{h}")
                if len(wq_tiles) < HEADS:
                    load_wq(len(wq_tiles))

            def qproj_step(nmm):
                """Emit nmm accumulating matmuls of the current head's
                projection; after the 16th, copy PSUM->qt_all on GpSimd."""
                h = q_state["h"]
                if h is None:
                    return
                wq_sb = wq_tiles[h]
                q_ps = q_state["ps"]
                for _ in range(nmm):
                    et = q_state["et"]
                    if et >= ET:
                        break
                    nc.tensor.matmul(q_ps[:], wq_sb[:, et, :],
                                     xt_sb[:, et, :],
                                     start=(et == 0), stop=(et == ET - 1))
                    q_state["et"] = et + 1
                if q_state["et"] >= ET:
                    with nc.allow_low_precision(reason="f16 q"):
                        nc.vector.tensor_copy(qt_all[:, h, :], q_ps[:])
                    q_state["h"] = None

            def o_tile(b, rt, ec, width=512, ec_off=0):
                """One output-projection tile: 16 accumulating matmuls from
                SBUF-resident outn/Wo into a qp-pool PSUM bank, then
                DVE copy + store."""
                o_ps = qp_ps.tile([128, R], F32, tag="qp",
                                  name=f"o_ps{b}{rt}{ec}{ec_off}")
                for ft in range(FT):
                    i0 = ft * NC_ROWS + rt * 128
                    nc.tensor.matmul(
                        o_ps[:, 0:width], outn_all[:, b, i0:i0 + 128],
                        wo_sb[:, ft, ec * 512 + ec_off:
                              ec * 512 + ec_off + width],
                        start=(ft == 0), stop=(ft == FT - 1))
                o_sb = ostp.tile([128, 512], F32, tag="ost")
                nc.vector.tensor_copy(o_sb[:, 0:width], o_ps[:, 0:width])
                nc.sync.dma_start(
                    o_d[b * NC_ROWS + rt * 128:
                        b * NC_ROWS + (rt + 1) * 128,
                        ec * 512 + ec_off:ec * 512 + ec_off + width],
                    o_sb[:, 0:width])

            o_split = {}
            # head u+2's first 2 matmuls run as seam filler at unit u-1's
            # jg7 (where no other PE work exists); the remaining 14 spread
            # through unit u with the PSUM->SBUF copy still landing by jg5
            QP_STEPS = (4, 3, 3, 2, 2, 2, 0, 0)      # unit 0 (no pre-steps)
            QP_STEPS_CONT = (2, 3, 3, 2, 2, 2, 0, 0)  # units 1..13

            def unit_ctx(u):
                hp, b = u // 2, u % 2
                return (kt_sb[:, b, :],
                        qt_all[:, 2 * hp:2 * hp + 2,
                               b * NC_ROWS:(b + 1) * NC_ROWS])

            def sims(u, jg):
                ktb, qt_pair = unit_ctx(u)
                sg = sg_ps.tile([128, 1024], F32, tag="sg",
                                name=f"sg{u}_{jg}")
                for kk in range(2):
                    jt = jg * 2 + kk
                    if u == 0 and jg == 0:
                        # first sims of the kernel: split per head so the
                        # h0 half starts as soon as h0's q copy lands,
                        # without waiting for h1's copy on the other engine
                        for hh in range(2):
                            nc.tensor.matmul(
                                sg[:, kk * 512 + hh * 256:
                                   kk * 512 + (hh + 1) * 256],
                                ktb[:, jt * 128:(jt + 1) * 128],
                                qt_pair[:, hh, :],
                                start=True, stop=True)
                        continue
                    nc.tensor.matmul(
                        sg[:, kk * 512:(kk + 1) * 512],
                        ktb[:, jt * 128:(jt + 1) * 128],
                        qt_pair,
                        start=True, stop=True)
                return sg

            sg = None
            for u in range(HEADS):
                hp, b = u // 2, u % 2
                if u + 2 < HEADS:
                    qproj_start(u + 2)
                # Both heads of the pair processed together: every attention
                # matmul has a 512-wide fp16 moving operand laid out
                # [h2, i256].  PSUM start/stop groups are bank-granular, so
                # outT and the q-projection need separate banks.
                acc = acc_ps.tile([128, 512], F32, tag="acc")
                qt_pair = qt_all[:, 2 * hp:2 * hp + 2,
                                 b * NC_ROWS:(b + 1) * NC_ROWS]
                s1024 = rbp.tile([128, 1024], F16, tag="s128")

                # Software-pipelined across jgs AND units: the avs of jg
                # trail the sims of jg+1 (or of the next unit's jg 0) on
                # the PE stream, so exp->av and sim->exp dependency
                # latencies overlap PE work instead of serializing either
                # the jg loop or the unit seam.  Q-projection steps run
                # 3,3,2,2,2,2,1,1 so the head's PSUM->SBUF copy is emitted
                # a jg before the next unit's hoisted sims might need it.
                sg = sims(u, 0)
                for jg in range(JT // 2):
                    es = esp.tile([128, 1024], F16, tag="es")
                    nc.scalar.activation(
                        es[:], sg[:], mybir.ActivationFunctionType.Exp,
                        scale=SCALE)
                    if jg + 1 < JT // 2:
                        sg = sims(u, jg + 1)
                    qproj_step(QP_STEPS[jg])
                    # Units 14/15 have no q-projection left to interleave
                    # (ACT would pace them); fill PE with the first two
                    # phase-C batch-0 tiles.  At unit 14 only heads 0..13
                    # of batch-0 outn exist, so its tile accumulates
                    # ft 0..13 and unit 15 closes the group (hp7's outn
                    # lands during unit 15's first jgs).
                    if u == 14 and jg in (1, 3):
                        if jg == 1:
                            o_split["ps"] = qp_ps.tile([128, R], F32,
                                                       tag="qp",
                                                       name="o_ps_sp")
                        for ft in range(7 * (jg - 1) // 2,
                                        7 * (jg + 1) // 2):
                            nc.tensor.matmul(
                                o_split["ps"][:],
                                outn_all[:, 0, ft * NC_ROWS:
                                         ft * NC_ROWS + 128],
                                wo_sb[:, ft, 0:512],
                                start=(ft == 0), stop=False)
                    if u == 15 and jg == 2:
                        o_ps = o_split["ps"]
                        for ft in (14, 15):
                            nc.tensor.matmul(
                                o_ps[:], outn_all[:, 0, ft * NC_ROWS:
                                                  ft * NC_ROWS + 128],
                                wo_sb[:, ft, 0:512],
                                start=False, stop=(ft == 15))
                        o_sb = ostp.tile([128, 512], F32, tag="ost",
                                         name="o_sb_sp")
                        nc.vector.tensor_copy(o_sb[:], o_ps[:])
                        nc.sync.dma_start(o_d[0:128, 0:512], o_sb[:])
                    if u == 15 and jg == 5:
                        o_tile(0, 1, 0)
                    if u == 15 and jg == 7:
                        # one more batch-0 tile: covers the wait for unit
                        # 15's softmax tail before phase C's batch-1 tiles
                        o_tile(0, 0, 1)
                    # softmax denominators: fp16 partial row-sums on DVE
                    # (2x mode; the 128-way partition reduction is on GpSimd
                    # below)
                    with nc.allow_low_precision(reason="f16 rowsum"):
                        if jg == 0:
                            nc.vector.tensor_copy(s1024[:], es[:])
                        else:
                            nc.vector.tensor_add(s1024[:], s1024[:], es[:])
                    for kk in range(2):
                        jt = jg * 2 + kk
                        esk = es[:, kk * 512:(kk + 1) * 512]
                        nc.tensor.matmul(acc[:], v_sb[:, b, jt, :],
                                         esk, start=(jt == 0),
                                         stop=(jt == JT - 1))
                # Wo prefetch: 2 ft-tiles per unit during units 0..7
                if u < 8:
                    for ft in (2 * u, 2 * u + 1):
                        nc.sync.dma_start(wo_sb[:, ft, :], wo_r[:, ft, :])
                # softmax-denominator tail, entirely off the PE stream:
                # DVE fold (fp16->fp32) -> GpSimd partition all-reduce ->
                # DVE reciprocal -> DVE normalize (fp32 acc * rb -> fp16)
                s512 = rbp.tile([128, 512], F32, tag="s512", bufs=1)
                sB = rbp.tile([128, 512], F32, tag="sB", bufs=1)
                rb_sb = rbp.tile([128, 512], F32, tag="rbs")
                with nc.allow_low_precision(reason="fold to f32"):
                    nc.vector.tensor_add(s512[:], s1024[:, 0:512],
                                         s1024[:, 512:1024])
                    nc.gpsimd.partition_all_reduce(
                        sB[:], s512[:], channels=128,
                        reduce_op=bass_isa.ReduceOp.add)
                    nc.vector.reciprocal(rb_sb[:], sB[:])
                    nc.vector.tensor_mul(
                        outn_all[:, b, 2 * hp * NC_ROWS:
                                 (2 * hp + 2) * NC_ROWS],
                        acc[:], rb_sb[:])

          # ---- Phase C: output projection (Wo already in SBUF; the first
          # two batch-0 tiles were emitted inside unit 15) ----
          for ec in range(4):
                for b in range(B):
                    for rt in range(2):
                        if ec == 0 and b == 0:
                            continue  # emitted in units 14/15
                        if ec == 1 and b == 0 and rt == 0:
                            continue  # emitted in unit 15
                        if ec == 3 and b == 1 and rt == 1:
                            continue  # final tile split below
                        o_tile(b, rt, ec)
          # final tile split (256/128/128) so the closing copy+store tail
          # is short
          o_tile(1, 1, 3, width=256, ec_off=0)
          o_tile(1, 1, 3, width=128, ec_off=256)
          o_tile(1, 1, 3, width=128, ec_off=384)


def _get_nc(reps: int = 1):
    if reps not in _CACHE:
        _CACHE[reps] = _build(reps)
    return _CACHE[reps]


def _make_in_maps(x, k, v, Wq, Wo):
    kt = np.ascontiguousarray(k.transpose(0, 2, 1)).astype(np.float16)
    # v as [p, b, jt, d]: per-partition DMA lines are jt*d contiguous
    v_c = np.ascontiguousarray(
        v.reshape(B, JT, 128, DH).transpose(2, 0, 1, 3)).astype(np.float16)
    # wq as [p, h, et, f]: per-head loads are et*f contiguous per partition
    wq = np.ascontiguousarray(
        Wq.reshape(ET, 128, HEADS, 128).transpose(1, 2, 0, 3)
    ).astype(np.float16)
    wo = np.ascontiguousarray(Wo).astype(np.float16)
    in_maps = []
    for c in range(NCORES):
        xs = x[:, c * NC_ROWS:(c + 1) * NC_ROWS, :]
        xt = np.ascontiguousarray(
            np.concatenate([xs[0].T, xs[1].T], axis=1)).astype(np.float16)
        in_maps.append({"xt": xt, "kt": kt, "v": v_c, "wq": wq, "wo": wo})
    return in_maps


def run_on_device(x, k, v, Wq, Wo, reps: int = 1):
    nc = _get_nc(reps)
    in_maps = _make_in_maps(x, k, v, Wq, Wo)
    res = run_bass_kernel_spmd(nc, in_maps, list(range(NCORES)))
    parts = [res.results[c]["o"].reshape(B, NC_ROWS, E) for c in range(NCORES)]
    return np.concatenate(parts, axis=1)


def kernel(x, k, v, Wq, Wo):
    x = np.asarray(x, dtype=np.float32)
    k = np.asarray(k, dtype=np.float32)
    v = np.asarray(v, dtype=np.float32)
    Wq = np.asarray(Wq, dtype=np.float32)
    Wo = np.asarray(Wo, dtype=np.float32)
    return run_on_device(x, k, v, Wq, Wo, reps=1)
